# revision 22
# baseline (speedup 1.0000x reference)
"""Trainium2 Bass kernel for nn_MeanEmbedding (fused gather + masked mean).

Strategy:
  out[b] = (1/len_b) * sum_{l < len_b} W[xs[b, l]]
         = (1/len_b) * sum_{v in U} count[v, b] * W[v]

The host builds the set U of unique masked token ids, the (tiny) count
matrix, and a COMPACTED bf16 table holding exactly the unique rows in
use, split evenly across the 8 cores.  Each core then just streams its
dense [128, R*1024] compacted shard from HBM with plain HWDGE DMAs (no
indices, no GPSIMD descriptor generation — which profiling showed is
slower than the DMA engines themselves for row-gathers) and reduces it
into per-sample sums with PE matmuls (lhsT = counts tile [128, B], rhs
= streamed rows, accumulated in PSUM).  The host sums the 8 per-core
partials and divides by the lengths.

Precision: the table rides as bf16 (2 KiB/row); per-element bf16
rounding (~2^-9 relative) keeps the output norm error ~1.7e-3, well
inside the 2e-2 gate, and halves HBM traffic vs fp32.  Counts ride as
bf16 too (exact for integers <= 256; larger counts are split host-side).

The stream is chunked (small head chunk so the PE starts early, big
body chunks for few instructions, small tail chunks for a short drain)
and double-buffered so the DMA engines never idle.
"""

import sys

sys.path.insert(0, "/opt/trn_rl_repo")

import ml_dtypes
import numpy as np

BF16 = ml_dtypes.bfloat16

B = 64
L = 2048
V = 50257
D = 1024
N_CORES = 8
P = 128

_program_cache = {}
LAST_RESULTS = None


def _chunk_schedule(R):
    """Tiles per DMA chunk: small head (fast PE start), big body, small
    tail (short drain)."""
    if R <= 4:
        return [1] * R
    head = [2, 4] if R > 6 else [min(2, R - 1)]
    tail = [3, 2, 1] if R > 12 else [1]
    rem = R - sum(head) - sum(tail)
    if rem < 0:
        return [2] * (R // 2) + [1] * (R % 2)
    body = []
    while rem > 0:
        c = min(8, rem)
        body.append(c)
        rem -= c
    return head + body + tail


def _build_program(R):
    """Build + compile the SPMD Bass program for R row-tiles per core."""
    import concourse.tile as tile
    from concourse import bacc, mybir

    nc = bacc.Bacc(
        "TRN2",
        target_bir_lowering=False,
        debug=False,
        enable_asserts=False,
        enable_partition_id=False,
        monotonic_sem_count=0,
        num_devices=N_CORES,
    )
    # compacted table: tile t, partition p holds unique row t*128+p
    table = nc.dram_tensor(
        "table", [P, R * D], mybir.dt.bfloat16, kind="ExternalInput"
    ).ap()
    counts = nc.dram_tensor(
        "counts", [P, R * B], mybir.dt.bfloat16, kind="ExternalInput"
    ).ap()
    out = nc.dram_tensor("out", [B, D], mybir.dt.float32, kind="ExternalOutput").ap()

    sched = _chunk_schedule(R)
    cmax = max(sched)

    with tile.TileContext(nc) as tc:
        with tc.tile_pool(name="meta", bufs=1) as meta, tc.tile_pool(
            name="strm", bufs=6
        ) as spool, tc.tile_pool(name="acc", bufs=1, space="PSUM") as psum, tc.tile_pool(
            name="outp", bufs=1
        ) as outp:
            counts_sb = meta.tile([P, R * B], mybir.dt.bfloat16)
            acc0 = psum.tile([B, 512], mybir.dt.float32)
            acc1 = psum.tile([B, 512], mybir.dt.float32)

            # interleave the first counts chunks between the first table
            # chunks on the sync engine so early matmuls unblock fast.
            n_cchunks = 4
            cchunk = -(-R // n_cchunks) * B
            cload = [
                (k * cchunk, min((k + 1) * cchunk, R * B)) for k in range(n_cchunks)
            ]
            cload = [(lo_, hi_) for lo_, hi_ in cload if lo_ < hi_]

            t0 = 0
            for i, c in enumerate(sched):
                ts = spool.tile([P, cmax * D], mybir.dt.bfloat16, tag="ts")
                nc.sync.dma_start(
                    ts[:, : c * D], table[:, t0 * D : (t0 + c) * D]
                )
                if i == 0:
                    for lo_, hi_ in cload[:1]:
                        nc.sync.dma_start(counts_sb[:, lo_:hi_], counts[:, lo_:hi_])
                elif i == 1:
                    for lo_, hi_ in cload[1:]:
                        nc.sync.dma_start(counts_sb[:, lo_:hi_], counts[:, lo_:hi_])
                for j in range(c):
                    t = t0 + j
                    lhsT = counts_sb[:, t * B : (t + 1) * B]
                    first, last = t == 0, t == R - 1
                    nc.tensor.matmul(
                        out=acc0[:], lhsT=lhsT, rhs=ts[:, j * D : j * D + 512],
                        start=first, stop=last,
                    )
                    nc.tensor.matmul(
                        out=acc1[:], lhsT=lhsT, rhs=ts[:, j * D + 512 : (j + 1) * D],
                        start=first, stop=last,
                    )
                t0 += c
            assert t0 == R

            # drain: copy each PSUM bank on its own engine, then the two
            # out-DMAs go via different DGE engines (sync / scalar) so
            # neither issue nor transfer serializes.
            res = outp.tile([B, D], mybir.dt.float32)
            nc.vector.tensor_copy(res[:, 0:512], acc0[:])
            nc.sync.dma_start(out[:, 0:512], res[:, 0:512])
            nc.scalar.copy(res[:, 512:1024], acc1[:])
            nc.scalar.dma_start(out[:, 512:1024], res[:, 512:1024])

    nc.compile()
    return nc


def _get_program(R):
    if R not in _program_cache:
        _program_cache[R] = _build_program(R)
    return _program_cache[R]


def _prep_inputs(xs, xs_len, W):
    """Host index preprocessing -> (R, per-core in_maps)."""
    mask = np.arange(L)[None, :] < xs_len.astype(np.int64)[:, None]
    toks = xs[mask].astype(np.int64)
    samp = np.broadcast_to(np.arange(B)[:, None], (B, L))[mask]
    U, inv = np.unique(toks, return_inverse=True)
    nU = len(U)
    cnt = np.bincount(inv * B + samp, minlength=nU * B).reshape(nU, B)
    # counts ride as bf16, exact only for integers <= 256; if any count is
    # larger (essentially impossible for random data), split that unique row
    # into several duplicate entries whose counts are each <= 256.
    if cnt.max() > 256:
        reps = -(-int(cnt.max()) // 256)
        U_l, cnt_l = [U], [np.minimum(cnt, 256)]
        rem = cnt - cnt_l[0]
        for _ in range(1, reps):
            rows = np.where(rem.max(axis=1) > 0)[0]
            take = np.minimum(rem[rows], 256)
            U_l.append(U[rows])
            cnt_l.append(take)
            rem[rows] -= take
        U = np.concatenate(U_l)
        cnt = np.concatenate(cnt_l, axis=0)
        nU = len(U)
    assert cnt.max() <= 256

    Wb = W.astype(BF16)  # [V, D] bf16

    # contiguous even split of the unique rows across cores
    q = -(-nU // N_CORES)
    R = max(1, -(-q // P))
    Npad = R * P

    in_maps = []
    for c in range(N_CORES):
        lo, hi = c * q, min((c + 1) * q, nU)
        n = max(0, hi - lo)
        rows = np.zeros((Npad, D), dtype=BF16)
        cnt_c = np.zeros((Npad, B), np.float32)
        if n > 0:
            rows[:n] = Wb[U[lo:hi]]
            cnt_c[:n] = cnt[lo:hi]
        # tile t, partition p <-> entry t*128+p
        table_c = np.ascontiguousarray(
            rows.reshape(R, P, D).transpose(1, 0, 2).reshape(P, R * D)
        )
        cnt_prb = np.ascontiguousarray(
            cnt_c.reshape(R, P, B).transpose(1, 0, 2).reshape(P, R * B)
        ).astype(BF16)
        in_maps.append({"table": table_c, "counts": cnt_prb})
    return R, in_maps


def kernel(xs, xs_len, embed_weight):
    global LAST_RESULTS
    import os
    from concourse import bass_utils

    xs = np.asarray(xs)
    xs_len = np.asarray(xs_len)
    W = np.ascontiguousarray(np.asarray(embed_weight, dtype=np.float32))
    assert xs.shape == (B, L) and W.shape == (V, D)

    R, in_maps = _prep_inputs(xs, xs_len, W)

    nc = _get_program(R)
    trace = bool(os.environ.get("MEANEMB_TRACE"))
    LAST_RESULTS = bass_utils.run_bass_kernel_spmd(
        nc, in_maps, core_ids=list(range(N_CORES)), trace=trace
    )

    partial = np.stack([LAST_RESULTS.results[c]["out"] for c in range(N_CORES)])
    total = partial.sum(axis=0)
    out = total / xs_len.astype(np.float32)[:, None]
    return out.astype(np.float32)
